# revision 6
# baseline (speedup 1.0000x reference)
"""MetaPathConnector kernel for Trainium2 (8 NeuronCores, Bass/Tile), v5.

Row-shards N=16384 nodes across 8 cores (2048 rows each), rotated featT so
each core's own rows occupy columns [0, 2048) (identical SPMD program).

v5 vs v4 (818us):
 - PE-side index packing: for PACK blocks, the PE accumulates onto each
   sims 512-chunk three rank-1 matmuls (+6144; -6144,+3; +jhi,+jlo) so the
   PSUM value becomes  p = (v quantized to 2^-11) + 3 + j*2^-22  with the
   11-bit block-local column j sitting exactly in mantissa bits 0..10.
   One DVE MAX8 per block then yields values AND indices (no FIND_INDEX8).
 - SCAN blocks keep max8+find_index8; their candidates are converted to the
   same packed format with two small [128,8] DVE ops.  PACK/SCAN split is
   tunable to balance the PE and DVE queues.
 - Gather via hardware-DGE indirect_dma_start (per-partition [128,1] row
   offsets) instead of the Q7 software dma_gather + idx replication DMAs.
 - Weighted-sum apply moved to the Pool engine (tensor_tensor with
   stride-0 broadcast weights); exp/normalize on ACT; DVE keeps only the
   scans, refine, and tiny index math.
 - lhs of the sims matmul is nrmT itself (v = 0.25*cos), so prep never
   materializes a separate projT operand; normalization is folded into a
   diag-scaled transpose matmul (rhs = pnat_chunk.T @ diag(0.5/|proj|)).
"""

from contextlib import ExitStack

import numpy as np
import ml_dtypes

import concourse.bass as bass
import concourse.mybir as mybir
import concourse.tile as tile
from concourse import bacc
from concourse.bass_utils import run_bass_kernel_spmd
from concourse.masks import make_identity

FP32 = mybir.dt.float32
BF16 = mybir.dt.bfloat16
I32 = mybir.dt.int32
U32 = mybir.dt.uint32
AF = mybir.ActivationFunctionType
ALU = mybir.AluOpType

N_NODES = 16384
D = 128
N_CORES = 8
K = 10
STRENGTH = 0.1
BLK = 2048
MMW = 512
NEG_DIAG = -8.0
QBASE = 6144.0           # quantize base: [4096,8192) binade, ulp 2^-11
POFF = 3.0               # packed offset: p = vq + 3 in [2,4), ulp 2^-22
PACK_BLOCKS = (True, True, True, True, True, True, False, False)


def build_nc(n_nodes=N_NODES, rows=N_NODES // N_CORES, n_cores=N_CORES,
             debug=False):
    nc = bacc.Bacc("TRN2", target_bir_lowering=False, debug=debug,
                   num_devices=n_cores)
    featT = nc.dram_tensor("featT", [D, n_nodes], BF16, kind="ExternalInput")
    feat_rows = nc.dram_tensor("feat_rows", [rows, D], FP32,
                               kind="ExternalInput")
    WT = nc.dram_tensor("WT", [D, D], BF16, kind="ExternalInput")
    jrhs = nc.dram_tensor("jrhs", [2, BLK + MMW], BF16,
                          kind="ExternalInput")
    out_rows = nc.dram_tensor("out_rows", [rows, D], FP32,
                              kind="ExternalOutput")
    projdram = nc.dram_tensor("projdram", [n_nodes, D], BF16)

    with tile.TileContext(nc) as tc, ExitStack() as ctx:
        _build(ctx, tc, featT.ap(), feat_rows.ap(), WT.ap(), jrhs.ap(),
               out_rows.ap(), projdram.ap(), n_nodes, rows)
    nc.compile()
    return nc


def _build(ctx, tc, featT, feat_rows, WT, jrhs, out_rows, projdram,
           n_nodes, rows):
    nc = tc.nc
    n_blocks = n_nodes // BLK          # 8
    n_tiles = rows // 128              # 16
    CW = n_blocks * 8                  # 64 candidates per row

    consts = ctx.enter_context(tc.tile_pool(name="consts", bufs=1))
    bigbuf = ctx.enter_context(tc.tile_pool(name="bigbuf", bufs=1))
    stream = ctx.enter_context(tc.tile_pool(name="stream", bufs=2))
    small = ctx.enter_context(tc.tile_pool(name="small", bufs=3))
    perts = ctx.enter_context(tc.tile_pool(name="perts", bufs=1))
    gpool = ctx.enter_context(tc.tile_pool(name="gpool", bufs=3))
    apool = ctx.enter_context(tc.tile_pool(name="apool", bufs=3))
    psum_blk = ctx.enter_context(
        tc.tile_pool(name="psum_blk", bufs=2, space="PSUM"))

    # ---------------- constants ----------------
    ident32 = consts.tile([128, 128], FP32)
    make_identity(nc, ident32[:])
    ident = consts.tile([128, 128], BF16)
    nc.vector.tensor_copy(ident[:], ident32[:])
    negI = consts.tile([128, 128], BF16)
    nc.gpsimd.memset(negI[:], 0.0)
    nc.gpsimd.affine_select(
        out=negI[:], in_=negI[:], compare_op=ALU.not_equal, fill=NEG_DIAG,
        base=0, pattern=[[-1, 128]], channel_multiplier=1)

    ones1 = consts.tile([1, 128], BF16)
    nc.gpsimd.memset(ones1[:], 1.0)
    ones2 = consts.tile([2, 128], BF16)
    nc.gpsimd.memset(ones2[:], 1.0)
    base_rhs = consts.tile([1, MMW], BF16)       # +12288
    nc.gpsimd.memset(base_rhs[:], QBASE)
    # jrhs cols [0,2048): (jhi, jlo); cols [2048,2560): (-12288, +6)
    jrhs_sb = consts.tile([2, BLK + MMW], BF16)
    nc.sync.dma_start(out=jrhs_sb[:], in_=jrhs)
    unbase_rhs = jrhs_sb[:, BLK:BLK + MMW]

    allbits = consts.tile([128, 1], I32)         # 0xFFFFFFFF
    nc.gpsimd.memset(allbits[:], -1.0)
    mask7ff = consts.tile([128, 1], I32)         # 0x7FF
    nc.gpsimd.memset(mask7ff[:], 2047.0)
    m8c = consts.tile([128, 1], I32)             # ~7
    nc.gpsimd.memset(m8c[:], -8.0)
    sh8c = consts.tile([128, 1], I32)            # shift 8
    nc.gpsimd.memset(sh8c[:], 8.0)
    negu = consts.tile([128, 1], FP32)           # -2^-22
    nc.gpsimd.memset(negu[:], -(2.0 ** -22))
    bias24 = consts.tile([128, 1], FP32)         # -4*POFF (ACT Exp bias)
    nc.gpsimd.memset(bias24[:], -4.0 * POFF)

    WT_sb = consts.tile([D, D], BF16)
    nc.sync.dma_start(out=WT_sb[:], in_=WT)

    # ---------------- prep ----------------
    pnat = bigbuf.tile([128, n_nodes], BF16)
    nrmT = bigbuf.tile([128, n_nodes], BF16)
    projdram_v = projdram.rearrange("(c p) d -> p c d", p=128)
    nchunks = n_nodes // 128           # 128
    ssq = perts.tile([128, nchunks], FP32)

    for b in range(n_blocks):
        fT = stream.tile([128, BLK], BF16, tag="ftblk")
        nc.sync.dma_start(out=fT[:], in_=featT[:, b * BLK:(b + 1) * BLK])
        pA = psum_blk.tile([128, BLK], FP32, tag="blk")
        for m in range(BLK // MMW):
            nc.tensor.matmul(pA[:, m * MMW:(m + 1) * MMW], lhsT=WT_sb[:],
                             rhs=fT[:, m * MMW:(m + 1) * MMW], start=True,
                             stop=True)
        ptmp = stream.tile([128, BLK], BF16, tag="ptmp")
        nc.scalar.copy(ptmp[:], pA[:])
        # transpose to natural rows, 512 at a time (bf16 transpose)
        for gg in range(4):
            g = 4 * b + gg             # global 512-group
            pT = psum_blk.tile([128, BLK], FP32, tag="blk")
            pG = pT[:, 0:256].bitcast(BF16)      # [128, 512] bf16 view
            for jj in range(4):
                c = 4 * g + jj
                nc.tensor.transpose(
                    pG[:, 128 * jj:128 * (jj + 1)],
                    ptmp[:, (gg * 4 + jj) * 128:(gg * 4 + jj + 1) * 128],
                    ident[:])
            nc.scalar.copy(pnat[:, g * 512:(g + 1) * 512], pG)
            nc.sync.dma_start(
                out=projdram_v[:, 4 * g:4 * (g + 1), :],
                in_=pnat[:, g * 512:(g + 1) * 512].rearrange(
                    "p (c d) -> p c d", d=128))
            sqt = small.tile([128, 512], FP32, tag="sqt")
            nc.vector.tensor_tensor(
                out=sqt[:], in0=pnat[:, g * 512:(g + 1) * 512],
                in1=pnat[:, g * 512:(g + 1) * 512], op=ALU.mult)
            nc.vector.tensor_reduce(
                ssq[:, 4 * g:4 * (g + 1)],
                sqt[:].rearrange("p (c d) -> p c d", d=128),
                axis=mybir.AxisListType.X, op=ALU.add)

    nrm = perts.tile([128, nchunks], FP32)
    nc.scalar.sqrt(nrm[:], ssq[:])
    rinv05 = perts.tile([128, nchunks], FP32)
    nc.vector.reciprocal(rinv05[:], nrm[:])
    nc.vector.tensor_scalar(out=rinv05[:], in0=rinv05[:], scalar1=0.5,
                            scalar2=None, op0=ALU.mult)

    def nrm_quad(q):
        """Build nrmT columns [512q, 512q+512) via diag-scaled transposes."""
        pT = psum_blk.tile([128, BLK], FP32, tag="blk")
        pG = pT[:, 1024:1536]
        for jj in range(4):
            c = 4 * q + jj
            dg = small.tile([128, 128], BF16, tag="diag")
            nc.vector.tensor_scalar(out=dg[:], in0=ident[:],
                                    scalar1=rinv05[:, c:c + 1],
                                    scalar2=None, op0=ALU.mult)
            nc.tensor.matmul(pG[:, 128 * jj:128 * (jj + 1)],
                             lhsT=pnat[:, c * 128:(c + 1) * 128],
                             rhs=dg[:], start=True, stop=True)
        nc.scalar.copy(nrmT[:, q * 512:(q + 1) * 512], pG)

    # ---------------- main-loop state ----------------
    E_all = perts.tile([128, n_tiles * K], FP32)
    gidx32 = perts.tile([128, n_tiles * K], I32)
    C_t = [perts.tile([128, CW], FP32, name=f"candA{i}") for i in range(2)]
    P16_t = [perts.tile([128, 16], FP32, name=f"p16A{i}") for i in range(2)]
    slot_t = [perts.tile([128, 16], U32, name=f"slotA{i}") for i in range(2)]
    G_tiles = [None] * n_tiles

    def topk_block(t, b):
        lhs = nrmT[:, t * 128:(t + 1) * 128]
        ps = psum_blk.tile([128, BLK], FP32, tag="blk")
        packed = PACK_BLOCKS[b]
        for m in range(BLK // MMW):
            is_diag_chunk = (b == 0 and m == (t * 128) // MMW)
            sl = ps[:, m * MMW:(m + 1) * MMW]
            rhs = nrmT[:, b * BLK + m * MMW:b * BLK + (m + 1) * MMW]
            if packed:
                nc.tensor.matmul(sl, lhsT=lhs, rhs=rhs, start=True,
                                 stop=False)
                if is_diag_chunk:
                    off = (t * 128) % BLK
                    nc.tensor.matmul(ps[:, off:off + 128], lhsT=negI[:],
                                     rhs=ident[:], start=False, stop=False,
                                     skip_group_check=True)
                nc.tensor.matmul(sl, lhsT=ones1[:], rhs=base_rhs[:],
                                 start=False, stop=False,
                                 skip_group_check=True)
                nc.tensor.matmul(sl, lhsT=ones2[:], rhs=unbase_rhs,
                                 start=False, stop=False,
                                 skip_group_check=True)
                nc.tensor.matmul(sl, lhsT=ones2[:],
                                 rhs=jrhs_sb[:, m * MMW:(m + 1) * MMW],
                                 start=False, stop=True,
                                 skip_group_check=True)
            else:
                nc.tensor.matmul(sl, lhsT=lhs, rhs=rhs, start=True,
                                 stop=not is_diag_chunk)
                if is_diag_chunk:
                    off = (t * 128) % BLK
                    nc.tensor.matmul(ps[:, off:off + 128], lhsT=negI[:],
                                     rhs=ident[:], start=False, stop=True,
                                     skip_group_check=True)
        C = C_t[t % 2]
        if packed:
            nc.vector.max(out=C[:, b * 8:(b + 1) * 8], in_=ps[:])
        else:
            v8 = small.tile([128, 8], FP32, tag="v8")
            nc.vector.max(out=v8[:], in_=ps[:])
            cidx = small.tile([128, 8], U32, tag="cidx")
            nc.vector.max_index(out=cidx[:], in_max=v8[:], in_values=ps[:])
            # quantize to the packed format: fl(fl(v+12288)-12282) | idx
            q8 = small.tile([128, 8], FP32, tag="q8")
            nc.vector.tensor_scalar(out=q8[:], in0=v8[:], scalar1=QBASE,
                                    scalar2=-(QBASE - POFF), op0=ALU.add,
                                    op1=ALU.add)
            nc.vector.scalar_tensor_tensor(
                out=C[:, b * 8:(b + 1) * 8].bitcast(I32),
                in0=q8[:].bitcast(I32), scalar=allbits[:, 0:1],
                in1=cidx[:].bitcast(I32),
                op0=ALU.bitwise_and, op1=ALU.bitwise_or)

    def refine_tile(t):
        C = C_t[t % 2]
        P16 = P16_t[t % 2]
        slot = slot_t[t % 2]
        nc.vector.max(out=P16[:, 0:8], in_=C[:])
        nc.vector.max_index(out=slot[:, 0:8], in_max=P16[:, 0:8],
                            in_values=C[:])
        C2 = small.tile([128, CW], FP32, tag="c2")
        nc.vector.match_replace(out=C2[:], in_to_replace=P16[:, 0:8],
                                in_values=C[:], imm_value=-3.0e38)
        nc.vector.max(out=P16[:, 8:16], in_=C2[:])
        nc.vector.max_index(out=slot[:, 8:16], in_max=P16[:, 8:16],
                            in_values=C2[:])

        # gidx = (slot>>3)*2048 | (pbits & 0x7FF)
        loc = small.tile([128, K], I32, tag="loc")
        nc.vector.scalar_tensor_tensor(
            out=loc[:], in0=P16[:, 0:K].bitcast(I32),
            scalar=mask7ff[:, 0:1], in1=loc[:],
            op0=ALU.bitwise_and, op1=ALU.bypass)
        base = small.tile([128, K], I32, tag="base")
        nc.vector.scalar_tensor_tensor(
            out=base[:], in0=slot[:, 0:K].bitcast(I32), scalar=m8c[:, 0:1],
            in1=base[:], op0=ALU.bitwise_and, op1=ALU.bypass)
        nc.vector.scalar_tensor_tensor(
            out=gidx32[:, t * K:(t + 1) * K], in0=base[:],
            scalar=sh8c[:, 0:1], in1=loc[:],
            op0=ALU.logical_shift_left, op1=ALU.bitwise_or)

        # E = exp(4*(p - j*2^-21) - 24), Z accumulated; then normalize by 1/Z
        jf = small.tile([128, K], FP32, tag="jf")
        nc.vector.tensor_copy(jf[:], loc[:])
        pin = small.tile([128, K], FP32, tag="pin")
        nc.vector.scalar_tensor_tensor(
            out=pin[:], in0=jf[:], scalar=negu[:, 0:1], in1=P16[:, 0:K],
            op0=ALU.mult, op1=ALU.add)
        E10 = E_all[:, t * K:(t + 1) * K]
        Z = small.tile([128, 1], FP32, tag="zz")
        nc.scalar.activation(E10, pin[:], AF.Exp, bias=bias24[:, 0:1],
                             scale=4.0, accum_out=Z[:])
        invZ = small.tile([128, 1], FP32, tag="iz")
        nc.vector.reciprocal(invZ[:], Z[:])
        nc.scalar.activation(E10, E10, AF.Copy, scale=invZ[:, 0:1])

    def issue_gather(t):
        G = gpool.tile([128, K, D], BF16, tag="gath")
        for k in range(K):
            nc.gpsimd.indirect_dma_start(
                out=G[:, k, :], out_offset=None, in_=projdram,
                in_offset=bass.IndirectOffsetOnAxis(
                    ap=gidx32[:, t * K + k:t * K + k + 1], axis=0))
        G_tiles[t] = G

    def issue_apply(t):
        G = G_tiles[t]
        acc = apool.tile([128, D], FP32, tag="acc")
        Eb = E_all[:, t * K:(t + 1) * K]
        nc.gpsimd.tensor_tensor(
            out=acc[:], in0=G[:, 0, :].squeeze(),
            in1=Eb[:, 0:1].to_broadcast([128, D]), op=ALU.mult)
        tmp = apool.tile([128, D], FP32, tag="tmp")
        for k in range(1, K):
            nc.gpsimd.tensor_tensor(
                out=tmp[:], in0=G[:, k, :].squeeze(),
                in1=Eb[:, k:k + 1].to_broadcast([128, D]), op=ALU.mult)
            nc.gpsimd.tensor_tensor(out=acc[:], in0=acc[:], in1=tmp[:],
                                    op=ALU.add)
        ft = apool.tile([128, D], FP32, tag="ft")
        nc.sync.dma_start(out=ft[:],
                          in_=feat_rows[t * 128:(t + 1) * 128, :])
        o = apool.tile([128, D], FP32, tag="oo")
        nc.gpsimd.tensor_tensor(out=o[:], in0=acc[:], in1=ft[:], op=ALU.add)
        nc.sync.dma_start(out=out_rows[t * 128:(t + 1) * 128, :], in_=o[:])

    # ---- weave: nrmT quads with tile-0 topk blocks ----
    for q in range(n_nodes // 512):    # 32 quads
        nrm_quad(q)
        if q % 4 == 3 and q >= 7:
            topk_block(0, (q - 7) // 4)
    topk_block(0, 7)
    refine_tile(0)
    issue_gather(0)

    for t in range(1, n_tiles):
        for b in range(n_blocks):
            topk_block(t, b)
        refine_tile(t)
        issue_gather(t)
        if t >= 2:
            issue_apply(t - 2)
    issue_apply(n_tiles - 2)
    issue_apply(n_tiles - 1)


_NC_CACHE = {}


def _get_nc(n_nodes, rows, n_cores):
    key = (n_nodes, rows, n_cores)
    if key not in _NC_CACHE:
        _NC_CACHE[key] = build_nc(n_nodes, rows, n_cores)
    return _NC_CACHE[key]


def make_in_maps(feat, W, emb, n_cores=N_CORES):
    n = feat.shape[0]
    rows = n // n_cores
    featT = np.ascontiguousarray(feat.T.astype(ml_dtypes.bfloat16))
    WT = np.ascontiguousarray((W.T * STRENGTH).astype(ml_dtypes.bfloat16))
    emb = np.ascontiguousarray(emb.astype(np.float32))
    j = np.arange(BLK)
    jrhs = np.zeros((2, BLK + MMW), dtype=ml_dtypes.bfloat16)
    jrhs[0, :BLK] = ((j >> 4).astype(np.float64) * 2.0 ** -18).astype(
        ml_dtypes.bfloat16)
    jrhs[1, :BLK] = ((j & 15).astype(np.float64) * 2.0 ** -22).astype(
        ml_dtypes.bfloat16)
    jrhs[0, BLK:] = ml_dtypes.bfloat16(-QBASE)
    jrhs[1, BLK:] = ml_dtypes.bfloat16(POFF)
    maps = []
    for c in range(n_cores):
        maps.append({
            "featT": np.ascontiguousarray(np.roll(featT, -rows * c, axis=1)),
            "feat_rows": np.ascontiguousarray(
                feat[rows * c:rows * (c + 1)].astype(np.float32)
                + STRENGTH * emb.astype(np.float32)),
            "WT": WT,
            "jrhs": jrhs,
        })
    return maps


def kernel(feat, W, emb):
    feat = np.asarray(feat, dtype=np.float32)
    W = np.asarray(W, dtype=np.float32)
    emb = np.asarray(emb, dtype=np.float32)
    n = feat.shape[0]
    rows = n // N_CORES
    nc = _get_nc(n, rows, N_CORES)
    in_maps = make_in_maps(feat, W, emb, N_CORES)
    res = run_bass_kernel_spmd(nc, in_maps, core_ids=list(range(N_CORES)))
    out = np.concatenate([res.results[c]["out_rows"] for c in range(N_CORES)],
                         axis=0)
    return out.astype(np.float32)


# revision 7
# speedup vs baseline: 1.3595x; 1.3595x over previous
"""MetaPathConnector kernel for Trainium2 (8 NeuronCores, Bass/Tile), v5.

Row-shards N=16384 nodes across 8 cores (2048 rows each), rotated featT so
each core's own rows occupy columns [0, 2048) (identical SPMD program).

v5 vs v4 (818us):
 - PE-side index packing: for PACK blocks, the PE accumulates onto each
   sims 512-chunk three rank-1 matmuls (+6144; -6144,+3; +jhi,+jlo) so the
   PSUM value becomes  p = (v quantized to 2^-11) + 3 + j*2^-22  with the
   11-bit block-local column j sitting exactly in mantissa bits 0..10.
   One DVE MAX8 per block then yields values AND indices (no FIND_INDEX8).
 - SCAN blocks keep max8+find_index8; their candidates are converted to the
   same packed format with two small [128,8] DVE ops.  PACK/SCAN split is
   tunable to balance the PE and DVE queues.
 - Gather via hardware-DGE indirect_dma_start (per-partition [128,1] row
   offsets) instead of the Q7 software dma_gather + idx replication DMAs.
 - Weighted-sum apply moved to the Pool engine (tensor_tensor with
   stride-0 broadcast weights); exp/normalize on ACT; DVE keeps only the
   scans, refine, and tiny index math.
 - lhs of the sims matmul is nrmT itself (v = 0.25*cos), so prep never
   materializes a separate projT operand; normalization is folded into a
   diag-scaled transpose matmul (rhs = pnat_chunk.T @ diag(0.5/|proj|)).
"""

from contextlib import ExitStack

import numpy as np
import ml_dtypes

import concourse.bass as bass
import concourse.mybir as mybir
import concourse.tile as tile
from concourse import bacc
from concourse.bass_utils import run_bass_kernel_spmd
from concourse.masks import make_identity

FP32 = mybir.dt.float32
BF16 = mybir.dt.bfloat16
I32 = mybir.dt.int32
U32 = mybir.dt.uint32
AF = mybir.ActivationFunctionType
ALU = mybir.AluOpType

N_NODES = 16384
D = 128
N_CORES = 8
K = 10
STRENGTH = 0.1
BLK = 2048
MMW = 512
NEG_DIAG = -8.0
QBASE = 6144.0           # quantize base: [4096,8192) binade, ulp 2^-11
POFF = 3.0               # packed offset: p = vq + 3 in [2,4), ulp 2^-22
PACK_BLOCKS = (False,) * 8


def build_nc(n_nodes=N_NODES, rows=N_NODES // N_CORES, n_cores=N_CORES,
             debug=False):
    nc = bacc.Bacc("TRN2", target_bir_lowering=False, debug=debug,
                   num_devices=n_cores)
    featT = nc.dram_tensor("featT", [D, n_nodes], BF16, kind="ExternalInput")
    feat_rows = nc.dram_tensor("feat_rows", [rows, D], FP32,
                               kind="ExternalInput")
    WT = nc.dram_tensor("WT", [D, D], BF16, kind="ExternalInput")
    jrhs = nc.dram_tensor("jrhs", [2, BLK + MMW], BF16,
                          kind="ExternalInput")
    out_rows = nc.dram_tensor("out_rows", [rows, D], FP32,
                              kind="ExternalOutput")
    projdram = nc.dram_tensor("projdram", [n_nodes, D], BF16)

    with tile.TileContext(nc) as tc, ExitStack() as ctx:
        _build(ctx, tc, featT.ap(), feat_rows.ap(), WT.ap(), jrhs.ap(),
               out_rows.ap(), projdram.ap(), n_nodes, rows)
    nc.compile()
    return nc


def _build(ctx, tc, featT, feat_rows, WT, jrhs, out_rows, projdram,
           n_nodes, rows):
    nc = tc.nc
    n_blocks = n_nodes // BLK          # 8
    n_tiles = rows // 128              # 16
    CW = n_blocks * 8                  # 64 candidates per row

    consts = ctx.enter_context(tc.tile_pool(name="consts", bufs=1))
    bigbuf = ctx.enter_context(tc.tile_pool(name="bigbuf", bufs=1))
    stream = ctx.enter_context(tc.tile_pool(name="stream", bufs=2))
    small = ctx.enter_context(tc.tile_pool(name="small", bufs=3))
    perts = ctx.enter_context(tc.tile_pool(name="perts", bufs=1))
    gpool = ctx.enter_context(tc.tile_pool(name="gpool", bufs=3))
    apool = ctx.enter_context(tc.tile_pool(name="apool", bufs=3))
    psum_blk = ctx.enter_context(
        tc.tile_pool(name="psum_blk", bufs=2, space="PSUM"))

    # ---------------- constants ----------------
    ident32 = consts.tile([128, 128], FP32)
    make_identity(nc, ident32[:])
    ident = consts.tile([128, 128], BF16)
    nc.vector.tensor_copy(ident[:], ident32[:])
    negI = consts.tile([128, 128], BF16)
    nc.gpsimd.memset(negI[:], 0.0)
    nc.gpsimd.affine_select(
        out=negI[:], in_=negI[:], compare_op=ALU.not_equal, fill=NEG_DIAG,
        base=0, pattern=[[-1, 128]], channel_multiplier=1)

    ones1 = consts.tile([1, 128], BF16)
    nc.gpsimd.memset(ones1[:], 1.0)
    ones2 = consts.tile([2, 128], BF16)
    nc.gpsimd.memset(ones2[:], 1.0)
    base_rhs = consts.tile([1, MMW], BF16)       # +12288
    nc.gpsimd.memset(base_rhs[:], QBASE)
    # jrhs cols [0,2048): (jhi, jlo); cols [2048,2560): (-12288, +6)
    jrhs_sb = consts.tile([2, BLK + MMW], BF16)
    nc.sync.dma_start(out=jrhs_sb[:], in_=jrhs)
    unbase_rhs = jrhs_sb[:, BLK:BLK + MMW]

    allbits = consts.tile([128, 1], I32)         # 0xFFFFFFFF
    nc.gpsimd.memset(allbits[:], -1.0)
    maskhi = consts.tile([128, 1], I32)          # ~0x7FF
    nc.gpsimd.memset(maskhi[:], -2048.0)
    mask7ff = consts.tile([128, 1], I32)         # 0x7FF
    nc.gpsimd.memset(mask7ff[:], 2047.0)
    m8c = consts.tile([128, 1], I32)             # ~7
    nc.gpsimd.memset(m8c[:], -8.0)
    sh8c = consts.tile([128, 1], I32)            # shift 8
    nc.gpsimd.memset(sh8c[:], 8.0)
    negu = consts.tile([128, 1], FP32)           # -2^-22
    nc.gpsimd.memset(negu[:], -(2.0 ** -22))
    bias24 = consts.tile([128, 1], FP32)         # -4*POFF (ACT Exp bias)
    nc.gpsimd.memset(bias24[:], -4.0 * POFF)

    WT_sb = consts.tile([D, D], BF16)
    nc.sync.dma_start(out=WT_sb[:], in_=WT)

    # ---------------- prep ----------------
    pnat = bigbuf.tile([128, n_nodes], BF16)
    nrmT = bigbuf.tile([128, n_nodes], BF16)
    projdram_v = projdram.rearrange("(c p) d -> p c d", p=128)
    nchunks = n_nodes // 128           # 128
    ssq = perts.tile([128, nchunks], FP32)

    for b in range(n_blocks):
        fT = stream.tile([128, BLK], BF16, tag="ftblk")
        nc.sync.dma_start(out=fT[:], in_=featT[:, b * BLK:(b + 1) * BLK])
        pA = psum_blk.tile([128, BLK], FP32, tag="blk")
        for m in range(BLK // MMW):
            nc.tensor.matmul(pA[:, m * MMW:(m + 1) * MMW], lhsT=WT_sb[:],
                             rhs=fT[:, m * MMW:(m + 1) * MMW], start=True,
                             stop=True)
        ptmp = stream.tile([128, BLK], BF16, tag="ptmp")
        nc.scalar.copy(ptmp[:], pA[:])
        # transpose to natural rows, 512 at a time (bf16 transpose)
        for gg in range(4):
            g = 4 * b + gg             # global 512-group
            pT = psum_blk.tile([128, BLK], FP32, tag="blk")
            pG = pT[:, 0:256].bitcast(BF16)      # [128, 512] bf16 view
            for jj in range(4):
                c = 4 * g + jj
                nc.tensor.transpose(
                    pG[:, 128 * jj:128 * (jj + 1)],
                    ptmp[:, (gg * 4 + jj) * 128:(gg * 4 + jj + 1) * 128],
                    ident[:])
            nc.scalar.copy(pnat[:, g * 512:(g + 1) * 512], pG)
            nc.sync.dma_start(
                out=projdram_v[:, 4 * g:4 * (g + 1), :],
                in_=pnat[:, g * 512:(g + 1) * 512].rearrange(
                    "p (c d) -> p c d", d=128))
            sqt = small.tile([128, 512], FP32, tag="sqt")
            nc.vector.tensor_tensor(
                out=sqt[:], in0=pnat[:, g * 512:(g + 1) * 512],
                in1=pnat[:, g * 512:(g + 1) * 512], op=ALU.mult)
            nc.vector.tensor_reduce(
                ssq[:, 4 * g:4 * (g + 1)],
                sqt[:].rearrange("p (c d) -> p c d", d=128),
                axis=mybir.AxisListType.X, op=ALU.add)

    nrm = perts.tile([128, nchunks], FP32)
    nc.scalar.sqrt(nrm[:], ssq[:])
    rinv05 = perts.tile([128, nchunks], FP32)
    nc.vector.reciprocal(rinv05[:], nrm[:])
    nc.vector.tensor_scalar(out=rinv05[:], in0=rinv05[:], scalar1=0.5,
                            scalar2=None, op0=ALU.mult)

    def nrm_quad(q):
        """Build nrmT columns [512q, 512q+512) via diag-scaled transposes."""
        pT = psum_blk.tile([128, BLK], FP32, tag="blk")
        pG = pT[:, 1024:1536]
        for jj in range(4):
            c = 4 * q + jj
            dg = small.tile([128, 128], BF16, tag="diag")
            nc.vector.tensor_scalar(out=dg[:], in0=ident[:],
                                    scalar1=rinv05[:, c:c + 1],
                                    scalar2=None, op0=ALU.mult)
            nc.tensor.matmul(pG[:, 128 * jj:128 * (jj + 1)],
                             lhsT=pnat[:, c * 128:(c + 1) * 128],
                             rhs=dg[:], start=True, stop=True)
        nc.scalar.copy(nrmT[:, q * 512:(q + 1) * 512], pG)

    # ---------------- main-loop state ----------------
    E_all = perts.tile([128, n_tiles * K], FP32)
    gidx32 = perts.tile([128, n_tiles * K], I32)
    C_t = [perts.tile([128, CW], FP32, name=f"candA{i}") for i in range(2)]
    P16_t = [perts.tile([128, 16], FP32, name=f"p16A{i}") for i in range(2)]
    slot_t = [perts.tile([128, 16], U32, name=f"slotA{i}") for i in range(2)]
    G_tiles = [None] * n_tiles

    def topk_block(t, b):
        lhs = nrmT[:, t * 128:(t + 1) * 128]
        ps = psum_blk.tile([128, BLK], FP32, tag="blk")
        packed = PACK_BLOCKS[b]
        for m in range(BLK // MMW):
            is_diag_chunk = (b == 0 and m == (t * 128) // MMW)
            sl = ps[:, m * MMW:(m + 1) * MMW]
            rhs = nrmT[:, b * BLK + m * MMW:b * BLK + (m + 1) * MMW]
            if packed:
                nc.tensor.matmul(sl, lhsT=lhs, rhs=rhs, start=True,
                                 stop=False)
                if is_diag_chunk:
                    off = (t * 128) % BLK
                    nc.tensor.matmul(ps[:, off:off + 128], lhsT=negI[:],
                                     rhs=ident[:], start=False, stop=False,
                                     skip_group_check=True)
                nc.tensor.matmul(sl, lhsT=ones1[:], rhs=base_rhs[:],
                                 start=False, stop=False,
                                 skip_group_check=True)
                nc.tensor.matmul(sl, lhsT=ones2[:], rhs=unbase_rhs,
                                 start=False, stop=False,
                                 skip_group_check=True)
                nc.tensor.matmul(sl, lhsT=ones2[:],
                                 rhs=jrhs_sb[:, m * MMW:(m + 1) * MMW],
                                 start=False, stop=True,
                                 skip_group_check=True)
            else:
                nc.tensor.matmul(sl, lhsT=lhs, rhs=rhs, start=True,
                                 stop=not is_diag_chunk)
                if is_diag_chunk:
                    off = (t * 128) % BLK
                    nc.tensor.matmul(ps[:, off:off + 128], lhsT=negI[:],
                                     rhs=ident[:], start=False, stop=True,
                                     skip_group_check=True)
        C = C_t[t % 2]
        if packed:
            nc.vector.max(out=C[:, b * 8:(b + 1) * 8], in_=ps[:])
        else:
            v8 = small.tile([128, 8], FP32, tag="v8")
            nc.vector.max(out=v8[:], in_=ps[:])
            cidx = small.tile([128, 8], U32, tag="cidx")
            nc.vector.max_index(out=cidx[:], in_max=v8[:], in_values=ps[:])
            # pack (value_bits & ~0x7FF) | local_idx  (ordering preserved)
            nc.vector.scalar_tensor_tensor(
                out=C[:, b * 8:(b + 1) * 8].bitcast(I32),
                in0=v8[:].bitcast(I32), scalar=maskhi[:, 0:1],
                in1=cidx[:].bitcast(I32),
                op0=ALU.bitwise_and, op1=ALU.bitwise_or)

    def refine_tile(t):
        C = C_t[t % 2]
        P16 = P16_t[t % 2]
        slot = slot_t[t % 2]
        nc.vector.max(out=P16[:, 0:8], in_=C[:])
        nc.vector.max_index(out=slot[:, 0:8], in_max=P16[:, 0:8],
                            in_values=C[:])
        C2 = small.tile([128, CW], FP32, tag="c2")
        nc.vector.match_replace(out=C2[:], in_to_replace=P16[:, 0:8],
                                in_values=C[:], imm_value=-3.0e38)
        nc.vector.max(out=P16[:, 8:16], in_=C2[:])
        nc.vector.max_index(out=slot[:, 8:16], in_max=P16[:, 8:16],
                            in_values=C2[:])

        # gidx = (slot>>3)*2048 | (pbits & 0x7FF)
        loc = small.tile([128, K], I32, tag="loc")
        nc.vector.scalar_tensor_tensor(
            out=loc[:], in0=P16[:, 0:K].bitcast(I32),
            scalar=mask7ff[:, 0:1], in1=loc[:],
            op0=ALU.bitwise_and, op1=ALU.bypass)
        base = small.tile([128, K], I32, tag="base")
        nc.vector.scalar_tensor_tensor(
            out=base[:], in0=slot[:, 0:K].bitcast(I32), scalar=m8c[:, 0:1],
            in1=base[:], op0=ALU.bitwise_and, op1=ALU.bypass)
        nc.vector.scalar_tensor_tensor(
            out=gidx32[:, t * K:(t + 1) * K], in0=base[:],
            scalar=sh8c[:, 0:1], in1=loc[:],
            op0=ALU.logical_shift_left, op1=ALU.bitwise_or)

        # E = exp(4 * masked_value) = exp(cos); Z accumulated
        pin = small.tile([128, K], FP32, tag="pin")
        nc.vector.scalar_tensor_tensor(
            out=pin[:].bitcast(I32), in0=P16[:, 0:K].bitcast(I32),
            scalar=maskhi[:, 0:1], in1=pin[:].bitcast(I32),
            op0=ALU.bitwise_and, op1=ALU.bypass)
        E10 = E_all[:, t * K:(t + 1) * K]
        Z = small.tile([128, 1], FP32, tag="zz")
        nc.scalar.activation(E10, pin[:], AF.Exp, scale=4.0,
                             accum_out=Z[:])
        invZ = small.tile([128, 1], FP32, tag="iz")
        nc.vector.reciprocal(invZ[:], Z[:])
        nc.scalar.activation(E10, E10, AF.Copy, scale=invZ[:, 0:1])

    def issue_gather(t):
        G = gpool.tile([128, K, D], BF16, tag="gath")
        for k in range(K):
            nc.gpsimd.indirect_dma_start(
                out=G[:, k, :], out_offset=None, in_=projdram,
                in_offset=bass.IndirectOffsetOnAxis(
                    ap=gidx32[:, t * K + k:t * K + k + 1], axis=0))
        G_tiles[t] = G

    def issue_apply(t):
        G = G_tiles[t]
        acc = apool.tile([128, D], FP32, tag="acc")
        Eb = E_all[:, t * K:(t + 1) * K]
        nc.gpsimd.tensor_tensor(
            out=acc[:], in0=G[:, 0, :].squeeze(),
            in1=Eb[:, 0:1].to_broadcast([128, D]), op=ALU.mult)
        tmp = apool.tile([128, D], FP32, tag="tmp")
        for k in range(1, K):
            nc.gpsimd.tensor_tensor(
                out=tmp[:], in0=G[:, k, :].squeeze(),
                in1=Eb[:, k:k + 1].to_broadcast([128, D]), op=ALU.mult)
            nc.gpsimd.tensor_tensor(out=acc[:], in0=acc[:], in1=tmp[:],
                                    op=ALU.add)
        ft = apool.tile([128, D], FP32, tag="ft")
        nc.sync.dma_start(out=ft[:],
                          in_=feat_rows[t * 128:(t + 1) * 128, :])
        o = apool.tile([128, D], FP32, tag="oo")
        nc.gpsimd.tensor_tensor(out=o[:], in0=acc[:], in1=ft[:], op=ALU.add)
        nc.sync.dma_start(out=out_rows[t * 128:(t + 1) * 128, :], in_=o[:])

    # ---- weave: nrmT quads with tile-0 topk blocks ----
    for q in range(n_nodes // 512):    # 32 quads
        nrm_quad(q)
        if q % 4 == 3 and q >= 7:
            topk_block(0, (q - 7) // 4)
    topk_block(0, 7)
    refine_tile(0)
    issue_gather(0)

    for t in range(1, n_tiles):
        for b in range(n_blocks):
            topk_block(t, b)
        refine_tile(t)
        issue_gather(t)
        if t >= 2:
            issue_apply(t - 2)
    issue_apply(n_tiles - 2)
    issue_apply(n_tiles - 1)


_NC_CACHE = {}


def _get_nc(n_nodes, rows, n_cores):
    key = (n_nodes, rows, n_cores)
    if key not in _NC_CACHE:
        _NC_CACHE[key] = build_nc(n_nodes, rows, n_cores)
    return _NC_CACHE[key]


def make_in_maps(feat, W, emb, n_cores=N_CORES):
    n = feat.shape[0]
    rows = n // n_cores
    featT = np.ascontiguousarray(feat.T.astype(ml_dtypes.bfloat16))
    WT = np.ascontiguousarray((W.T * STRENGTH).astype(ml_dtypes.bfloat16))
    emb = np.ascontiguousarray(emb.astype(np.float32))
    j = np.arange(BLK)
    jrhs = np.zeros((2, BLK + MMW), dtype=ml_dtypes.bfloat16)
    jrhs[0, :BLK] = ((j >> 4).astype(np.float64) * 2.0 ** -18).astype(
        ml_dtypes.bfloat16)
    jrhs[1, :BLK] = ((j & 15).astype(np.float64) * 2.0 ** -22).astype(
        ml_dtypes.bfloat16)
    jrhs[0, BLK:] = ml_dtypes.bfloat16(-QBASE)
    jrhs[1, BLK:] = ml_dtypes.bfloat16(POFF)
    maps = []
    for c in range(n_cores):
        maps.append({
            "featT": np.ascontiguousarray(np.roll(featT, -rows * c, axis=1)),
            "feat_rows": np.ascontiguousarray(
                feat[rows * c:rows * (c + 1)].astype(np.float32)
                + STRENGTH * emb.astype(np.float32)),
            "WT": WT,
            "jrhs": jrhs,
        })
    return maps


def kernel(feat, W, emb):
    feat = np.asarray(feat, dtype=np.float32)
    W = np.asarray(W, dtype=np.float32)
    emb = np.asarray(emb, dtype=np.float32)
    n = feat.shape[0]
    rows = n // N_CORES
    nc = _get_nc(n, rows, N_CORES)
    in_maps = make_in_maps(feat, W, emb, N_CORES)
    res = run_bass_kernel_spmd(nc, in_maps, core_ids=list(range(N_CORES)))
    out = np.concatenate([res.results[c]["out_rows"] for c in range(N_CORES)],
                         axis=0)
    return out.astype(np.float32)
